# revision 14
# baseline (speedup 1.0000x reference)
"""Trainium2 Bass kernel for nn_Aligner (location-sensitive attention + LSTM decoder).

Sharding (8 NeuronCores, SPMD, identical program + per-core host inputs):
  - LSTM gate dim (4*RNN=4096) tensor-parallel: core k computes a (64, 512)
    gate block [i|f|o|g] for its 128-slice of the rnn dim, batch stationary.
  - Attention batch-parallel: core k owns samples [8k, 8k+8).
  - Per step: AllGather of h2-slices (transposed) and of per-core context w.
  - sigma(x) = 0.5*tanh(x/2)+0.5 -> single ACT table set (exp+tanh).
    h is stored doubled (h2=2h); h-consuming weights pre-scaled by 0.5.
  - Matmuls run as float32r (full PE rate at N>=512); the conv/location and
    context paths run bf16 to halve the per-step im2col DMA.
  - All core-specific selection is via per-core host inputs (indicator
    matrices), so the compiled program is identical across cores.
"""
import json
import os

import numpy as np
import ml_dtypes

import concourse.bass as bass
import concourse.mybir as mybir
import concourse.tile as tile

B, T, D = 64, 512, 512
RNN, ATT, OUT = 1024, 128, 80
SPK_DIM, N_SPKRS = 64, 128
NF, KS = 32, 31
PAD = (KS - 1) // 2
NC_N = 8
BL = B // NC_N            # 8 samples/core
GS = 4 * RNN // NC_N      # 512 gate cols/core
BT = BL * T               # 4096
TPAD = T + 32             # 544 padded alpha row stride
NQ = 4                    # attention quarters
QW = BT // NQ             # 1024

f32 = mybir.dt.float32
bf16 = mybir.dt.bfloat16
f32r = mybir.dt.float32r
AF = mybir.ActivationFunctionType
ALU = mybir.AluOpType

_CACHE = {}


def _fix_bir_json(bir):
    """This walrus build allows at most one semaphore wait per instruction;
    hoist extras onto same-engine NoOps inserted just before."""
    j = json.loads(bir)
    n = [0]

    def fix_block(block):
        insts = block.get("instructions")
        if not insts:
            return
        out = []
        for ins in insts:
            waits = (ins.get("sync_info") or {}).get("on_wait") or []
            if len(waits) > 1:
                ins["sync_info"]["on_wait"] = waits[-1:]
                for w in waits[:-1]:
                    n[0] += 1
                    out.append({"engine": ins["engine"], "ins": [], "outs": [],
                                "name": f"I-mwfix-{n[0]}", "opcode": "NoOp",
                                "sync_info": {"on_wait": [w]}})
            out.append(ins)
        block["instructions"] = out

    def walk(o):
        if isinstance(o, dict):
            if isinstance(o.get("instructions"), list):
                fix_block(o)
            for v in o.values():
                walk(v)
        elif isinstance(o, list):
            for v in o:
                walk(v)

    walk(j)
    return json.dumps(j).encode()


def _install_hooks():
    if getattr(_install_hooks, "done", False):
        return
    _install_hooks.done = True
    import concourse.bass_utils as bu
    import concourse.bass2jax as b2j
    orig = bu.compile_bir_kernel

    def patched(bir_json, tmpdir, neff_name="file.neff"):
        if isinstance(bir_json, str):
            bir_json = bir_json.encode()
        return orig(_fix_bir_json(bir_json), tmpdir, neff_name=neff_name)

    bu.compile_bir_kernel = patched
    b2j.compile_bir_kernel = patched


def _build(n_steps):
    SKIP_IM2COL = os.environ.get("SKIP_IM2COL", "0") == "1"
    SKIP_AF = os.environ.get("SKIP_AF", "0") == "1"
    SKIP_CC = os.environ.get("SKIP_CC", "0") == "1"

    nc = bass.Bass("TRN2", target_bir_lowering=False, debug=False,
                   num_devices=NC_N)

    def din(name, shape, dt=f32):
        return nc.dram_tensor(name, shape, dt, kind="ExternalInput")

    x_wg = din("wg", [12 * 128, GS])
    x_b0 = din("b0", [B, GS])
    x_pm = din("pm", [ATT, BT])
    x_g = din("gmat", [62, ATT], bf16)
    x_ind = din("indic", [B, BT], bf16)     # rows: global sample one-hots
    x_idf = din("identf", [128, 128])
    x_idb = din("identb", [128, 128], bf16)
    x_wv = din("wvoh", [ATT, 8 * BL])
    x_wq = din("wqt", [RNN, ATT], bf16)
    x_wp = din("wpt", [12 * 128, OUT], bf16)
    x_bp = din("bpv", [1, OUT], bf16)
    x_on = din("onesb", [1, B], bf16)
    x_mm = din("memt", [128, BL * 4 * D], bf16)
    out_d = nc.dram_tensor("out", [B, n_steps, OUT], f32, kind="ExternalOutput")

    RG = [list(range(NC_N))]

    with tile.TileContext(nc) as tc:
        with (
            tc.tile_pool(name="const", bufs=1) as cst,
            tc.tile_pool(name="state", bufs=1) as st,
            tc.tile_pool(name="work", bufs=2) as wk,
            tc.tile_pool(name="psA", bufs=2, space="PSUM") as psA,
            tc.tile_pool(name="psB", bufs=1, space="PSUM") as psB,
            tc.tile_pool(name="dram", bufs=1, space="DRAM") as dram,
        ):
            # ---------------- constants
            wg = cst.tile([128, 12 * GS], f32r)
            for c in range(12):
                nc.sync.dma_start(wg[:, c * GS:(c + 1) * GS],
                                  x_wg[c * 128:(c + 1) * 128, :].bitcast(f32r))
            b0 = cst.tile([B, GS], f32r)
            nc.sync.dma_start(b0[:], x_b0[:].bitcast(f32r))
            pm = cst.tile([ATT, BT], f32r)
            nc.sync.dma_start(pm[:], x_pm[:].bitcast(f32r))
            idf = cst.tile([128, 128], f32)
            nc.sync.dma_start(idf[:], x_idf[:])
            idr = cst.tile([128, 128], f32r)
            nc.sync.dma_start(idr[:], x_idf[:].bitcast(f32r))
            idb = cst.tile([128, 128], bf16)
            nc.sync.dma_start(idb[:], x_idb[:])
            wv = cst.tile([ATT, 8 * BL], f32r)
            nc.sync.dma_start(wv[:], x_wv[:].bitcast(f32r))
            wq = cst.tile([128, 8 * ATT], bf16)
            for c in range(8):
                nc.sync.dma_start(wq[:, c * ATT:(c + 1) * ATT],
                                  x_wq[c * 128:(c + 1) * 128, :])
            wp = cst.tile([128, 12 * OUT], bf16)
            for c in range(12):
                nc.sync.dma_start(wp[:, c * OUT:(c + 1) * OUT],
                                  x_wp[c * 128:(c + 1) * 128, :])
            bpv = cst.tile([1, OUT], bf16)
            nc.sync.dma_start(bpv[:], x_bp[:])
            onb = cst.tile([1, B], bf16)
            nc.sync.dma_start(onb[:], x_on[:])
            mm = cst.tile([128, BL * 4 * D], bf16)
            nc.sync.dma_start(mm[:], x_mm[:])

            # conv operands: lhsT rows 0-63 pq (dynamic), 64-125 G (static)
            conv_lhs = st.tile([126, ATT], bf16)
            nc.sync.dma_start(conv_lhs[B:126, :], x_g[:])
            conv_rhs = st.tile([126, BT], bf16)
            nc.sync.dma_start(conv_rhs[0:B, :], x_ind[:])

            # ---------------- state
            hTg = st.tile([128, 8 * B], f32r)
            hTb = st.tile([128, 8 * B], bf16)
            wT = st.tile([128, 4 * B], f32r)
            wTb = st.tile([128, 4 * B], bf16)
            c_st = st.tile([B, 128], f32)
            a_cum = st.tile([BL, T], f32)
            af = st.tile([2, BL * TPAD], bf16)
            awm = st.tile([128, 32 * 32], bf16)
            th = st.tile([ATT, BT], f32r)
            for t_ in (c_st, a_cum, af, awm):
                nc.vector.memset(t_[:], 0.0)
            nc.vector.memset(hTb[:], 0.0)
            nc.vector.memset(wTb[:], 0.0)
            for t_ in (hTg, wT, th):
                nc.vector.memset(t_[:].bitcast(f32), 0.0)
            nc.vector.memset(conv_rhs[B:126, :], 0.0)
            nc.vector.memset(conv_lhs[0:B, :], 0.0)

            # ---------------- collective bounce buffers
            bh_in = dram.tile([128, B], f32)
            bh_out = dram.tile([NC_N * 128, B], f32)
            bw_in = dram.tile([BL, D], f32)
            bw_out = dram.tile([B, D], f32)

            # ---------------- steps
            for t in range(n_steps):
                # gates (B, GS): bias + w-part + h2-part
                ps_g = psB.tile([B, GS], f32, tag="gates")
                nc.tensor.matmul(ps_g[:], idr[0:B, 0:B], b0[:],
                                 start=True, stop=False)
                for c in range(8):
                    nc.tensor.matmul(ps_g[:], hTg[:, c * B:(c + 1) * B],
                                     wg[:, (4 + c) * GS:(5 + c) * GS],
                                     start=False, stop=False)
                for c in range(4):
                    nc.tensor.matmul(ps_g[:], wT[:, c * B:(c + 1) * B],
                                     wg[:, c * GS:(c + 1) * GS],
                                     start=False, stop=(c == 3))

                # pointwise LSTM (sigma via tanh half-angle, h2 = 2h)
                yifo = wk.tile([B, 384], f32, tag="yifo")
                nc.scalar.activation(yifo[:], ps_g[:, 0:384], AF.Tanh, scale=0.5)
                tg = wk.tile([B, 128], f32, tag="tg")
                nc.scalar.activation(tg[:], ps_g[:, 384:512], AF.Tanh)
                t1 = wk.tile([B, 128], f32, tag="t1")
                t2 = wk.tile([B, 128], f32, tag="t2")
                nc.vector.tensor_mul(t1[:], yifo[:, 128:256], c_st[:])
                nc.vector.tensor_add(t1[:], t1[:], c_st[:])
                nc.vector.tensor_mul(t2[:], yifo[:, 0:128], tg[:])
                nc.vector.tensor_add(t2[:], t2[:], tg[:])
                nc.vector.tensor_add(t1[:], t1[:], t2[:])      # 2*c_new
                nc.vector.tensor_scalar_mul(c_st[:], t1[:], 0.5)
                tct = wk.tile([B, 128], f32, tag="tct")
                nc.scalar.activation(tct[:], t1[:], AF.Tanh, scale=0.5)
                h2 = wk.tile([B, 128], f32, tag="h2")
                nc.vector.tensor_mul(h2[:], yifo[:, 256:384], tct[:])
                nc.vector.tensor_add(h2[:], h2[:], tct[:])

                # h2 -> transpose -> AllGather -> hTg
                ps_t = psB.tile([128, 256], f32, tag="misc")
                nc.tensor.transpose(ps_t[:, 0:B], h2[:], idf[0:B, 0:B])
                hto = wk.tile([128, B], f32, tag="hto")
                nc.vector.tensor_copy(hto[:], ps_t[:, 0:B])
                nc.sync.dma_start(bh_in[:], hto[:])
                if not SKIP_CC:
                    nc.gpsimd.collective_compute(
                        "AllGather", ALU.bypass, ins=[bh_in.opt()],
                        outs=[bh_out.opt()], replica_groups=RG)
                else:
                    for cc in range(8):
                        nc.sync.dma_start(bh_out[cc * 128:(cc + 1) * 128, :],
                                          bh_in[:])
                for c in range(8):
                    nc.sync.dma_start(
                        hTg[:, c * B:(c + 1) * B],
                        bh_out[c * 128:(c + 1) * 128, :].bitcast(f32r))
                nc.vector.tensor_copy(hTb[:], hTg[:].bitcast(f32))

                # pq for all 64 samples -> conv_lhs rows 0-63 (bf16)
                ps_pq = psB.tile([B, ATT], f32, tag="misc")
                for c in range(8):
                    nc.tensor.matmul(ps_pq[:], hTb[:, c * B:(c + 1) * B],
                                     wq[:, c * ATT:(c + 1) * ATT],
                                     start=(c == 0), stop=(c == 7))
                nc.vector.tensor_copy(conv_lhs[0:B, :], ps_pq[:])

                # attention: quarters of (ATT, QW) psum -> tanh -> th
                ps_e = psB.tile([BL, T], f32, tag="e")
                for q in range(NQ):
                    ps_q = psA.tile([ATT, QW], f32, tag="argq")
                    for c in range(QW // 512):
                        lo = c * 512
                        g_lo = q * QW + lo
                        nc.tensor.matmul(ps_q[:, lo:lo + 512], idr[:, 0:ATT],
                                         pm[:, g_lo:g_lo + 512],
                                         start=True, stop=False)
                        nc.tensor.matmul(ps_q[:, lo:lo + 512], conv_lhs[:],
                                         conv_rhs[:, g_lo:g_lo + 512],
                                         start=False, stop=True)
                    nc.scalar.activation(th[:, q * QW:(q + 1) * QW], ps_q[:],
                                         AF.Tanh)
                    for bl in (2 * q, 2 * q + 1):
                        nc.tensor.matmul(ps_e[:], wv[:, bl * BL:(bl + 1) * BL],
                                         th[:, bl * T:(bl + 1) * T],
                                         start=(bl == 0), stop=(bl == 7))

                # softmax (unnormalized exp + folded normalization)
                aw_b = wk.tile([BL, T], bf16, tag="awb")
                s_t = wk.tile([BL, 1], f32, tag="s")
                nc.scalar.activation(aw_b[:], ps_e[:], AF.Exp, accum_out=s_t[:])
                rs = wk.tile([BL, 1], f32, tag="rs")
                nc.vector.reciprocal(rs[:], s_t[:])
                aw_n = wk.tile([BL, T], bf16, tag="awn")
                nc.vector.tensor_scalar_mul(aw_n[:], aw_b[:], rs[:])
                nc.vector.scalar_tensor_tensor(
                    a_cum[:], aw_b[:], rs[:], a_cum[:], ALU.mult, ALU.add)
                acb = wk.tile([BL, T], bf16, tag="acb")
                nc.vector.tensor_copy(acb[:], a_cum[:])

                # alpha-flat rows + im2col for next step's conv
                af_r = af[:, :].rearrange("c (b p) -> c b p", b=BL)
                if not SKIP_AF:
                    nc.sync.dma_start(af_r[0:1, :, 16:16 + T], aw_n[:])
                    nc.sync.dma_start(af_r[1:2, :, 16:16 + T], acb[:])
                af_ap = af[:, :]
                for ci in range(2 if not SKIP_IM2COL else 0):
                    for kx in range(KS):
                        src = bass.AP(
                            tensor=af_ap.tensor,
                            offset=(af_ap.offset + ci * af_ap.ap[0][0]
                                    + kx + 1),
                            ap=[[af_ap.ap[0][0], 1], [TPAD, BL], [1, T]])
                        nc.sync.dma_start(
                            conv_rhs[B + ci * KS + kx:B + ci * KS + kx + 1, :],
                            src)

                # aw transposes -> masked diagonal blocks for w_new
                ps_at = psB.tile([128, 256], bf16, tag="misc2")
                for tc_i in range(4):
                    nc.tensor.transpose(ps_at[:, tc_i * BL:(tc_i + 1) * BL],
                                        aw_n[:, tc_i * 128:(tc_i + 1) * 128],
                                        idb[0:BL, 0:BL])
                awm_ap = awm[:, :]
                pa_ap = ps_at[:, 0:32]
                dst = bass.AP(tensor=awm_ap.tensor, offset=awm_ap.offset,
                              ap=[[awm_ap.ap[0][0], 128], [256, 4], [129, 2],
                                  [32, 4]])
                src = bass.AP(tensor=pa_ap.tensor, offset=pa_ap.offset,
                              ap=[[pa_ap.ap[0][0], 128], [1, 4], [4, 2],
                                  [8, 4]])
                nc.vector.tensor_copy(dst, src)

                # w_new: context vectors for own samples
                ps_w_full = psB.tile([128, D], f32, tag="gates")
                for g1 in range(2):
                    for tc_i in range(4):
                        for g0 in range(4):
                            j = g0 * 8 + g1 * 4 + tc_i
                            b_g = g1 * 4 + g0
                            nc.tensor.matmul(
                                ps_w_full[32 * g0:32 * g0 + 32, :],
                                awm[:, j * 32:(j + 1) * 32],
                                mm[:, (b_g * 4 + tc_i) * D:
                                   (b_g * 4 + tc_i + 1) * D],
                                start=(g1 == 0 and tc_i == 0),
                                stop=(g1 == 1 and tc_i == 3),
                                tile_position=(0, 32 * g0),
                                skip_group_check=True)
                w_own = wk.tile([128, D], f32, tag="wown")
                nc.vector.tensor_copy(w_own[:], ps_w_full[:])
                bw_ap = bw_in[:, :]
                for c in range(4):
                    dstw = bass.AP(tensor=bw_ap.tensor,
                                   offset=bw_ap.offset + c * D,
                                   ap=[[4 * D, 2], [1, D]])
                    nc.sync.dma_start(dstw, w_own[32 * c:32 * c + 2, :])
                if not SKIP_CC:
                    nc.gpsimd.collective_compute(
                        "AllGather", ALU.bypass, ins=[bw_in.opt()],
                        outs=[bw_out.opt()], replica_groups=RG)
                else:
                    for cc in range(8):
                        nc.sync.dma_start(bw_out[cc * BL:(cc + 1) * BL, :],
                                          bw_in[:])
                w_g = wk.tile([B, D], f32, tag="wg2")
                nc.sync.dma_start(w_g[:], bw_out[:])
                ps_wt = psB.tile([128, 256], f32, tag="misc")
                for c in range(4):
                    nc.tensor.transpose(ps_wt[:, c * B:(c + 1) * B],
                                        w_g[:, c * 128:(c + 1) * 128],
                                        idf[0:B, 0:B])
                nc.vector.tensor_copy(wT[:], ps_wt[:])
                nc.vector.tensor_copy(wTb[:], ps_wt[:])

                # out projection for all samples (host extracts own rows)
                ps_o = psB.tile([B, OUT], f32, tag="misc2")
                for c in range(4):
                    nc.tensor.matmul(ps_o[:], wTb[:, c * B:(c + 1) * B],
                                     wp[:, c * OUT:(c + 1) * OUT],
                                     start=(c == 0), stop=False)
                for c in range(8):
                    nc.tensor.matmul(ps_o[:], hTb[:, c * B:(c + 1) * B],
                                     wp[:, (4 + c) * OUT:(5 + c) * OUT],
                                     start=False, stop=False)
                nc.tensor.matmul(ps_o[:], onb[:], bpv[:],
                                 start=False, stop=True)
                o_sb = wk.tile([B, OUT], f32, tag="osb")
                nc.scalar.copy(o_sb[:], ps_o[:])
                nc.sync.dma_start(out_d[:, t, :], o_sb[:])

    return nc


# --------------------------------------------------------------- host side
def _prep_inputs(inputs):
    spkr = np.asarray(inputs["spkr"]).astype(np.int64)
    memory = np.asarray(inputs["memory"], np.float32)
    spk_emb = np.asarray(inputs["spkr_emb"], np.float32)
    Wq = np.asarray(inputs["Wq"], np.float32)
    Wm = np.asarray(inputs["Wm"], np.float32)
    Wv = np.asarray(inputs["Wv"], np.float32)
    conv_w = np.asarray(inputs["conv_w"], np.float32)
    loc_w = np.asarray(inputs["loc_w"], np.float32)
    W_ih = np.asarray(inputs["W_ih"], np.float32)
    W_hh = np.asarray(inputs["W_hh"], np.float32)
    b_ih = np.asarray(inputs["b_ih"], np.float32)
    b_hh = np.asarray(inputs["b_hh"], np.float32)
    Wp = np.asarray(inputs["Wp"], np.float32)
    bp = np.asarray(inputs["bp"], np.float32)

    spk_vec = spk_emb[spkr]                       # (B, SPK)
    pm_full = np.einsum("btd,ad->bta", memory, Wm).astype(np.float32)
    G = np.einsum("af,fck->ack", loc_w, conv_w)   # (ATT, 2, KS)
    gmat = np.transpose(G, (1, 2, 0)).reshape(2 * KS, ATT)  # [(c,k), a]

    identf = np.eye(128, dtype=np.float32)
    in_maps = []
    for k in range(NC_N):
        rows = np.concatenate([
            np.arange(128 * k, 128 * (k + 1)),            # i
            1024 + np.arange(128 * k, 128 * (k + 1)),     # f
            3072 + np.arange(128 * k, 128 * (k + 1)),     # o
            2048 + np.arange(128 * k, 128 * (k + 1)),     # g
        ])
        wcat = np.concatenate([W_ih[rows, :D], 0.5 * W_hh[rows, :]], axis=1)
        wg_h = np.ascontiguousarray(wcat.T)               # (1536, GS)
        b0 = (spk_vec @ W_ih[rows, D:D + SPK_DIM].T
              + b_ih[rows] + b_hh[rows]).astype(np.float32)
        own = slice(BL * k, BL * (k + 1))
        pm_k = np.ascontiguousarray(
            pm_full[own].reshape(BT, ATT).T)              # (ATT, BT)
        ind = np.zeros((B, BT), np.float32)
        for j in range(BL):
            ind[BL * k + j, j * T:(j + 1) * T] = 1.0
        wvoh = np.zeros((ATT, 8 * BL), np.float32)
        for j in range(BL):
            wvoh[:, j * BL + j] = Wv[0]
        wqt = np.ascontiguousarray((0.5 * Wq).T)          # (RNN, ATT)
        wpt = np.concatenate([Wp[:, RNN:], 0.5 * Wp[:, :RNN]],
                             axis=1).T.astype(np.float32)  # (1536, OUT)
        memt = np.ascontiguousarray(
            memory[own].reshape(BL, 4, 128, D).transpose(2, 0, 1, 3)
            .reshape(128, BL * 4 * D))
        in_maps.append({
            "wg": wg_h, "b0": b0, "pm": pm_k,
            "gmat": gmat.astype(ml_dtypes.bfloat16), "indic": ind.astype(ml_dtypes.bfloat16),
            "identf": identf, "identb": identf.astype(ml_dtypes.bfloat16),
            "wvoh": wvoh, "wqt": wqt.astype(ml_dtypes.bfloat16),
            "wpt": wpt.astype(ml_dtypes.bfloat16),
            "bpv": bp.reshape(1, OUT).astype(ml_dtypes.bfloat16),
            "memt": memt.astype(ml_dtypes.bfloat16),
            "onesb": np.ones((1, B), ml_dtypes.bfloat16),
        })
    return in_maps


def kernel(**inputs):
    _install_hooks()
    n_steps = int(np.asarray(inputs["output_timesteps"]))
    if n_steps not in _CACHE:
        _CACHE[n_steps] = _build(n_steps)
    nc = _CACHE[n_steps]
    in_maps = _prep_inputs(inputs)

    from concourse.bass_utils import run_bass_kernel_spmd as _run
    runner = globals().get("run_bass_kernel_spmd", _run)
    res = runner(nc, in_maps, core_ids=list(range(NC_N)))
    global _LAST_RESULT
    _LAST_RESULT = res
    outs = []
    for k in range(NC_N):
        outs.append(res.results[k]["out"][BL * k:BL * (k + 1)])  # (BL, S, OUT)
    full = np.concatenate(outs, axis=0)           # (B, S, OUT)
    return np.ascontiguousarray(full.transpose(0, 2, 1)).astype(np.float32)


# revision 16
# speedup vs baseline: 9.3132x; 9.3132x over previous
"""Trainium2 Bass kernel for nn_Aligner (location-sensitive attention + LSTM decoder).

Sharding (8 NeuronCores, SPMD, identical program + per-core host inputs):
  - LSTM gate dim (4*RNN=4096) tensor-parallel: core k computes a (64, 512)
    gate block [i|f|o|g] for its 128-slice of the rnn dim, batch stationary.
  - Attention batch-parallel: core k owns samples [8k, 8k+8).
  - Per step: AllGather of h2-slices (transposed) and of per-core context w.
  - sigma(x) = 0.5*tanh(x/2)+0.5 -> single ACT table set (exp+tanh).
    h is stored doubled (h2=2h); h-consuming weights pre-scaled by 0.5.
  - Matmuls run as float32r (full PE rate at N>=512); the conv/location and
    context paths run bf16 to halve the per-step im2col DMA.
  - All core-specific selection is via per-core host inputs (indicator
    matrices), so the compiled program is identical across cores.
"""
import json
import os

import numpy as np
import ml_dtypes

import concourse.bass as bass
import concourse.mybir as mybir
import concourse.tile as tile

B, T, D = 64, 512, 512
RNN, ATT, OUT = 1024, 128, 80
SPK_DIM, N_SPKRS = 64, 128
NF, KS = 32, 31
PAD = (KS - 1) // 2
NC_N = 8
BL = B // NC_N            # 8 samples/core
GS = 4 * RNN // NC_N      # 512 gate cols/core
BT = BL * T               # 4096
TPAD = T + 32             # 544 padded alpha row stride
NQ = 4                    # attention quarters
QW = BT // NQ             # 1024

f32 = mybir.dt.float32
bf16 = mybir.dt.bfloat16
f32r = mybir.dt.float32r
AF = mybir.ActivationFunctionType
ALU = mybir.AluOpType

_CACHE = {}


def _fix_bir_json(bir):
    """This walrus build allows at most one semaphore wait per instruction;
    hoist extras onto same-engine NoOps inserted just before."""
    j = json.loads(bir)
    n = [0]

    def fix_block(block):
        insts = block.get("instructions")
        if not insts:
            return
        out = []
        for ins in insts:
            waits = (ins.get("sync_info") or {}).get("on_wait") or []
            if len(waits) > 1:
                ins["sync_info"]["on_wait"] = waits[-1:]
                for w in waits[:-1]:
                    n[0] += 1
                    out.append({"engine": ins["engine"], "ins": [], "outs": [],
                                "name": f"I-mwfix-{n[0]}", "opcode": "NoOp",
                                "sync_info": {"on_wait": [w]}})
            out.append(ins)
        block["instructions"] = out

    def walk(o):
        if isinstance(o, dict):
            if isinstance(o.get("instructions"), list):
                fix_block(o)
            for v in o.values():
                walk(v)
        elif isinstance(o, list):
            for v in o:
                walk(v)

    walk(j)
    return json.dumps(j).encode()


def _install_hooks():
    if getattr(_install_hooks, "done", False):
        return
    _install_hooks.done = True
    import concourse.bass_utils as bu
    import concourse.bass2jax as b2j
    orig = bu.compile_bir_kernel

    def patched(bir_json, tmpdir, neff_name="file.neff"):
        if isinstance(bir_json, str):
            bir_json = bir_json.encode()
        return orig(_fix_bir_json(bir_json), tmpdir, neff_name=neff_name)

    bu.compile_bir_kernel = patched
    b2j.compile_bir_kernel = patched


def _build(n_steps):
    SKIP_IM2COL = os.environ.get("SKIP_IM2COL", "0") == "1"
    SKIP_AF = os.environ.get("SKIP_AF", "0") == "1"
    SKIP_CC = os.environ.get("SKIP_CC", "0") == "1"

    nc = bass.Bass("TRN2", target_bir_lowering=False, debug=False,
                   num_devices=NC_N)

    def din(name, shape, dt=f32):
        return nc.dram_tensor(name, shape, dt, kind="ExternalInput")

    x_wg = din("wg", [12 * 128, GS])
    x_b0 = din("b0", [B, GS])
    x_pm = din("pm", [ATT, BT])
    x_g = din("gmat", [62, ATT], bf16)
    x_ind = din("indic", [B, BT], bf16)     # rows: global sample one-hots
    x_idf = din("identf", [128, 128])
    x_idb = din("identb", [128, 128], bf16)
    x_wv = din("wvoh", [ATT, 8 * BL])
    x_wq = din("wqt", [RNN, ATT], bf16)
    x_wp = din("wpt", [12 * 128, OUT], bf16)
    x_bp = din("bpv", [1, OUT], bf16)
    x_on = din("onesb", [1, B], bf16)
    x_mm = din("memt", [128, BL * 4 * D], bf16)
    out_d = nc.dram_tensor("out", [B, n_steps, OUT], f32, kind="ExternalOutput")

    RG = [list(range(NC_N))]

    with tile.TileContext(nc) as tc:
        with (
            tc.tile_pool(name="const", bufs=1) as cst,
            tc.tile_pool(name="state", bufs=1) as st,
            tc.tile_pool(name="work", bufs=2) as wk,
            tc.tile_pool(name="psA", bufs=2, space="PSUM") as psA,
            tc.tile_pool(name="psB", bufs=1, space="PSUM") as psB,
            tc.tile_pool(name="dram", bufs=1, space="DRAM") as dram,
        ):
            # ---------------- constants
            wg = cst.tile([128, 12 * GS], f32r)
            for c in range(12):
                nc.sync.dma_start(wg[:, c * GS:(c + 1) * GS],
                                  x_wg[c * 128:(c + 1) * 128, :].bitcast(f32r))
            b0 = cst.tile([B, GS], f32r)
            nc.sync.dma_start(b0[:], x_b0[:].bitcast(f32r))
            pm = cst.tile([ATT, BT], f32r)
            nc.sync.dma_start(pm[:], x_pm[:].bitcast(f32r))
            idf = cst.tile([128, 128], f32)
            nc.sync.dma_start(idf[:], x_idf[:])
            idr = cst.tile([128, 128], f32r)
            nc.sync.dma_start(idr[:], x_idf[:].bitcast(f32r))
            idb = cst.tile([128, 128], bf16)
            nc.sync.dma_start(idb[:], x_idb[:])
            wv = cst.tile([ATT, 8 * BL], f32r)
            nc.sync.dma_start(wv[:], x_wv[:].bitcast(f32r))
            wq = cst.tile([128, 8 * ATT], bf16)
            for c in range(8):
                nc.sync.dma_start(wq[:, c * ATT:(c + 1) * ATT],
                                  x_wq[c * 128:(c + 1) * 128, :])
            wp = cst.tile([128, 12 * OUT], bf16)
            for c in range(12):
                nc.sync.dma_start(wp[:, c * OUT:(c + 1) * OUT],
                                  x_wp[c * 128:(c + 1) * 128, :])
            bpv = cst.tile([1, OUT], bf16)
            nc.sync.dma_start(bpv[:], x_bp[:])
            onb = cst.tile([1, B], bf16)
            nc.sync.dma_start(onb[:], x_on[:])
            mm = cst.tile([128, BL * 4 * D], bf16)
            nc.sync.dma_start(mm[:], x_mm[:])

            # conv operands: lhsT rows 0-63 pq (dynamic), 64-125 G (static)
            conv_lhs = st.tile([126, ATT], bf16)
            nc.sync.dma_start(conv_lhs[B:126, :], x_g[:])
            conv_rhs = st.tile([126, BT], bf16)
            nc.sync.dma_start(conv_rhs[0:B, :], x_ind[:])

            # ---------------- state
            hTg = st.tile([128, 8 * B], f32r)
            hTb = st.tile([128, 8 * B], bf16)
            wT = st.tile([128, 4 * B], f32r)
            wTb = st.tile([128, 4 * B], bf16)
            c_st = st.tile([B, 128], f32)
            a_cum = st.tile([BL, T], f32)
            af = st.tile([2, BL * TPAD], bf16)
            awm = st.tile([128, 32 * 32], bf16)
            th = st.tile([ATT, BT], f32r)
            for t_ in (c_st, a_cum, af, awm):
                nc.vector.memset(t_[:], 0.0)
            nc.vector.memset(hTb[:], 0.0)
            nc.vector.memset(wTb[:], 0.0)
            for t_ in (hTg, wT, th):
                nc.vector.memset(t_[:].bitcast(f32), 0.0)
            nc.vector.memset(conv_rhs[B:126, :], 0.0)
            nc.vector.memset(conv_lhs[0:B, :], 0.0)

            # ---------------- collective bounce buffers
            bh_in = dram.tile([128, B], f32)
            bh_out = dram.tile([NC_N * 128, B], f32)
            bw_in = dram.tile([BL, D], f32)
            bw_out = dram.tile([B, D], f32)
            baf = dram.tile([2, BL * TPAD], bf16)

            # ---------------- steps
            for t in range(n_steps):
                # gates (B, GS): bias + w-part + h2-part
                ps_g = psB.tile([B, GS], f32, tag="gates")
                nc.tensor.matmul(ps_g[:], idr[0:B, 0:B], b0[:],
                                 start=True, stop=False)
                for c in range(8):
                    nc.tensor.matmul(ps_g[:], hTg[:, c * B:(c + 1) * B],
                                     wg[:, (4 + c) * GS:(5 + c) * GS],
                                     start=False, stop=False)
                for c in range(4):
                    nc.tensor.matmul(ps_g[:], wT[:, c * B:(c + 1) * B],
                                     wg[:, c * GS:(c + 1) * GS],
                                     start=False, stop=(c == 3))

                # pointwise LSTM (sigma via tanh half-angle, h2 = 2h)
                yifo = wk.tile([B, 384], f32, tag="yifo")
                nc.scalar.activation(yifo[:], ps_g[:, 0:384], AF.Tanh, scale=0.5)
                tg = wk.tile([B, 128], f32, tag="tg")
                nc.scalar.activation(tg[:], ps_g[:, 384:512], AF.Tanh)
                t1 = wk.tile([B, 128], f32, tag="t1")
                t2 = wk.tile([B, 128], f32, tag="t2")
                nc.vector.tensor_mul(t1[:], yifo[:, 128:256], c_st[:])
                nc.vector.tensor_add(t1[:], t1[:], c_st[:])
                nc.vector.tensor_mul(t2[:], yifo[:, 0:128], tg[:])
                nc.vector.tensor_add(t2[:], t2[:], tg[:])
                nc.vector.tensor_add(t1[:], t1[:], t2[:])      # 2*c_new
                nc.vector.tensor_scalar_mul(c_st[:], t1[:], 0.5)
                tct = wk.tile([B, 128], f32, tag="tct")
                nc.scalar.activation(tct[:], t1[:], AF.Tanh, scale=0.5)
                h2 = wk.tile([B, 128], f32, tag="h2")
                nc.vector.tensor_mul(h2[:], yifo[:, 256:384], tct[:])
                nc.vector.tensor_add(h2[:], h2[:], tct[:])

                # h2 -> transpose -> AllGather -> hTg
                ps_t = psB.tile([128, 256], f32, tag="misc")
                nc.tensor.transpose(ps_t[:, 0:B], h2[:], idf[0:B, 0:B])
                hto = wk.tile([128, B], f32, tag="hto")
                nc.vector.tensor_copy(hto[:], ps_t[:, 0:B])
                nc.scalar.dma_start(bh_in[:], hto[:])
                if not SKIP_CC:
                    nc.gpsimd.collective_compute(
                        "AllGather", ALU.bypass, ins=[bh_in.opt()],
                        outs=[bh_out.opt()], replica_groups=RG)
                else:
                    for cc in range(8):
                        nc.sync.dma_start(bh_out[cc * 128:(cc + 1) * 128, :],
                                          bh_in[:])
                bho = bh_out[:, :].bitcast(f32r)
                srch = bass.AP(tensor=bho.tensor, offset=bho.offset,
                               ap=[[B, 128], [128 * B, 8], [1, B]])
                nc.sync.dma_start(hTg[:, :], srch)
                nc.vector.tensor_copy(hTb[:], hTg[:].bitcast(f32))

                # pq for all 64 samples -> conv_lhs rows 0-63 (bf16)
                ps_pq = psB.tile([B, ATT], f32, tag="misc")
                for c in range(8):
                    nc.tensor.matmul(ps_pq[:], hTb[:, c * B:(c + 1) * B],
                                     wq[:, c * ATT:(c + 1) * ATT],
                                     start=(c == 0), stop=(c == 7))
                nc.vector.tensor_copy(conv_lhs[0:B, :], ps_pq[:])

                # attention: quarters of (ATT, QW) psum -> tanh -> th
                ps_e = psB.tile([BL, T], f32, tag="e")
                for q in range(NQ):
                    ps_q = psA.tile([ATT, QW], f32, tag="argq")
                    for c in range(QW // 512):
                        lo = c * 512
                        g_lo = q * QW + lo
                        nc.tensor.matmul(ps_q[:, lo:lo + 512], idr[:, 0:ATT],
                                         pm[:, g_lo:g_lo + 512],
                                         start=True, stop=False)
                        nc.tensor.matmul(ps_q[:, lo:lo + 512], conv_lhs[:],
                                         conv_rhs[:, g_lo:g_lo + 512],
                                         start=False, stop=True)
                    nc.scalar.activation(th[:, q * QW:(q + 1) * QW], ps_q[:],
                                         AF.Tanh)
                    for bl in (2 * q, 2 * q + 1):
                        nc.tensor.matmul(ps_e[:], wv[:, bl * BL:(bl + 1) * BL],
                                         th[:, bl * T:(bl + 1) * T],
                                         start=(bl == 0), stop=(bl == 7))

                # softmax (unnormalized exp + folded normalization)
                aw_b = wk.tile([BL, T], bf16, tag="awb")
                s_t = wk.tile([BL, 1], f32, tag="s")
                nc.scalar.activation(aw_b[:], ps_e[:], AF.Exp, accum_out=s_t[:])
                rs = wk.tile([BL, 1], f32, tag="rs")
                nc.vector.reciprocal(rs[:], s_t[:])
                aw_n = wk.tile([BL, T], bf16, tag="awn")
                nc.vector.tensor_scalar_mul(aw_n[:], aw_b[:], rs[:])
                nc.vector.scalar_tensor_tensor(
                    a_cum[:], aw_b[:], rs[:], a_cum[:], ALU.mult, ALU.add)
                acb = wk.tile([BL, T], bf16, tag="acb")
                nc.vector.tensor_copy(acb[:], a_cum[:])

                # alpha-flat rows + im2col for next step's conv
                af_r = af[:, :].rearrange("c (b p) -> c b p", b=BL)
                if not SKIP_AF:
                    nc.gpsimd.dma_start(af_r[0:1, :, 16:16 + T], aw_n[:])
                    nc.scalar.dma_start(af_r[1:2, :, 16:16 + T], acb[:])
                if not SKIP_IM2COL:
                    nc.scalar.dma_start(baf[:], af[:])
                    baf_ap = baf[:, :]
                    for ci in range(2):
                        src = bass.AP(
                            tensor=baf_ap.tensor,
                            offset=baf_ap.offset + ci * BL * TPAD + 1,
                            ap=[[1, KS], [TPAD, BL], [1, T]])
                        nc.gpsimd.dma_start(
                            conv_rhs[B + ci * KS:B + (ci + 1) * KS, :], src)

                # aw transposes -> masked diagonal blocks for w_new
                ps_at = psB.tile([128, 256], bf16, tag="misc2")
                for tc_i in range(4):
                    nc.tensor.transpose(ps_at[:, tc_i * BL:(tc_i + 1) * BL],
                                        aw_n[:, tc_i * 128:(tc_i + 1) * 128],
                                        idb[0:BL, 0:BL])
                awm_ap = awm[:, :]
                pa_ap = ps_at[:, 0:32]
                dst = bass.AP(tensor=awm_ap.tensor, offset=awm_ap.offset,
                              ap=[[awm_ap.ap[0][0], 128], [256, 4], [129, 2],
                                  [32, 4]])
                src = bass.AP(tensor=pa_ap.tensor, offset=pa_ap.offset,
                              ap=[[pa_ap.ap[0][0], 128], [1, 4], [4, 2],
                                  [8, 4]])
                nc.vector.tensor_copy(dst, src)

                # w_new: context vectors for own samples
                ps_w_full = psB.tile([128, D], f32, tag="gates")
                for g1 in range(2):
                    for tc_i in range(4):
                        for g0 in range(4):
                            j = g0 * 8 + g1 * 4 + tc_i
                            b_g = g1 * 4 + g0
                            nc.tensor.matmul(
                                ps_w_full[32 * g0:32 * g0 + 32, :],
                                awm[:, j * 32:(j + 1) * 32],
                                mm[:, (b_g * 4 + tc_i) * D:
                                   (b_g * 4 + tc_i + 1) * D],
                                start=(g1 == 0 and tc_i == 0),
                                stop=(g1 == 1 and tc_i == 3),
                                tile_position=(0, 32 * g0),
                                skip_group_check=True)
                w_own = wk.tile([128, D], f32, tag="wown")
                nc.vector.tensor_copy(w_own[:], ps_w_full[:])
                bw_ap = bw_in[:, :]
                for c in range(4):
                    dstw = bass.AP(tensor=bw_ap.tensor,
                                   offset=bw_ap.offset + c * D,
                                   ap=[[4 * D, 2], [1, D]])
                    (nc.sync if c % 2 == 0 else nc.scalar).dma_start(
                        dstw, w_own[32 * c:32 * c + 2, :])
                if not SKIP_CC:
                    nc.gpsimd.collective_compute(
                        "AllGather", ALU.bypass, ins=[bw_in.opt()],
                        outs=[bw_out.opt()], replica_groups=RG)
                else:
                    for cc in range(8):
                        nc.sync.dma_start(bw_out[cc * BL:(cc + 1) * BL, :],
                                          bw_in[:])
                w_g = wk.tile([B, D], f32, tag="wg2")
                nc.gpsimd.dma_start(w_g[:], bw_out[:])
                ps_wt = psB.tile([128, 256], f32, tag="misc")
                for c in range(4):
                    nc.tensor.transpose(ps_wt[:, c * B:(c + 1) * B],
                                        w_g[:, c * 128:(c + 1) * 128],
                                        idf[0:B, 0:B])
                nc.vector.tensor_copy(wT[:], ps_wt[:])
                nc.vector.tensor_copy(wTb[:], ps_wt[:])

                # out projection for all samples (host extracts own rows)
                ps_o = psB.tile([B, OUT], f32, tag="misc2")
                for c in range(4):
                    nc.tensor.matmul(ps_o[:], wTb[:, c * B:(c + 1) * B],
                                     wp[:, c * OUT:(c + 1) * OUT],
                                     start=(c == 0), stop=False)
                for c in range(8):
                    nc.tensor.matmul(ps_o[:], hTb[:, c * B:(c + 1) * B],
                                     wp[:, (4 + c) * OUT:(5 + c) * OUT],
                                     start=False, stop=False)
                nc.tensor.matmul(ps_o[:], onb[:], bpv[:],
                                 start=False, stop=True)
                o_sb = wk.tile([B, OUT], f32, tag="osb")
                nc.scalar.copy(o_sb[:], ps_o[:])
                nc.gpsimd.dma_start(out_d[:, t, :], o_sb[:])

    return nc


# --------------------------------------------------------------- host side
def _prep_inputs(inputs):
    spkr = np.asarray(inputs["spkr"]).astype(np.int64)
    memory = np.asarray(inputs["memory"], np.float32)
    spk_emb = np.asarray(inputs["spkr_emb"], np.float32)
    Wq = np.asarray(inputs["Wq"], np.float32)
    Wm = np.asarray(inputs["Wm"], np.float32)
    Wv = np.asarray(inputs["Wv"], np.float32)
    conv_w = np.asarray(inputs["conv_w"], np.float32)
    loc_w = np.asarray(inputs["loc_w"], np.float32)
    W_ih = np.asarray(inputs["W_ih"], np.float32)
    W_hh = np.asarray(inputs["W_hh"], np.float32)
    b_ih = np.asarray(inputs["b_ih"], np.float32)
    b_hh = np.asarray(inputs["b_hh"], np.float32)
    Wp = np.asarray(inputs["Wp"], np.float32)
    bp = np.asarray(inputs["bp"], np.float32)

    spk_vec = spk_emb[spkr]                       # (B, SPK)
    pm_full = np.einsum("btd,ad->bta", memory, Wm).astype(np.float32)
    G = np.einsum("af,fck->ack", loc_w, conv_w)   # (ATT, 2, KS)
    gmat = np.transpose(G, (1, 2, 0)).reshape(2 * KS, ATT)  # [(c,k), a]

    identf = np.eye(128, dtype=np.float32)
    in_maps = []
    for k in range(NC_N):
        rows = np.concatenate([
            np.arange(128 * k, 128 * (k + 1)),            # i
            1024 + np.arange(128 * k, 128 * (k + 1)),     # f
            3072 + np.arange(128 * k, 128 * (k + 1)),     # o
            2048 + np.arange(128 * k, 128 * (k + 1)),     # g
        ])
        wcat = np.concatenate([W_ih[rows, :D], 0.5 * W_hh[rows, :]], axis=1)
        wg_h = np.ascontiguousarray(wcat.T)               # (1536, GS)
        b0 = (spk_vec @ W_ih[rows, D:D + SPK_DIM].T
              + b_ih[rows] + b_hh[rows]).astype(np.float32)
        own = slice(BL * k, BL * (k + 1))
        pm_k = np.ascontiguousarray(
            pm_full[own].reshape(BT, ATT).T)              # (ATT, BT)
        ind = np.zeros((B, BT), np.float32)
        for j in range(BL):
            ind[BL * k + j, j * T:(j + 1) * T] = 1.0
        wvoh = np.zeros((ATT, 8 * BL), np.float32)
        for j in range(BL):
            wvoh[:, j * BL + j] = Wv[0]
        wqt = np.ascontiguousarray((0.5 * Wq).T)          # (RNN, ATT)
        wpt = np.concatenate([Wp[:, RNN:], 0.5 * Wp[:, :RNN]],
                             axis=1).T.astype(np.float32)  # (1536, OUT)
        memt = np.ascontiguousarray(
            memory[own].reshape(BL, 4, 128, D).transpose(2, 0, 1, 3)
            .reshape(128, BL * 4 * D))
        in_maps.append({
            "wg": wg_h, "b0": b0, "pm": pm_k,
            "gmat": gmat.astype(ml_dtypes.bfloat16), "indic": ind.astype(ml_dtypes.bfloat16),
            "identf": identf, "identb": identf.astype(ml_dtypes.bfloat16),
            "wvoh": wvoh, "wqt": wqt.astype(ml_dtypes.bfloat16),
            "wpt": wpt.astype(ml_dtypes.bfloat16),
            "bpv": bp.reshape(1, OUT).astype(ml_dtypes.bfloat16),
            "memt": memt.astype(ml_dtypes.bfloat16),
            "onesb": np.ones((1, B), ml_dtypes.bfloat16),
        })
    return in_maps


def kernel(**inputs):
    _install_hooks()
    n_steps = int(np.asarray(inputs["output_timesteps"]))
    if n_steps not in _CACHE:
        _CACHE[n_steps] = _build(n_steps)
    nc = _CACHE[n_steps]
    in_maps = _prep_inputs(inputs)

    from concourse.bass_utils import run_bass_kernel_spmd as _run
    runner = globals().get("run_bass_kernel_spmd", _run)
    res = runner(nc, in_maps, core_ids=list(range(NC_N)))
    global _LAST_RESULT
    _LAST_RESULT = res
    outs = []
    for k in range(NC_N):
        outs.append(res.results[k]["out"][BL * k:BL * (k + 1)])  # (BL, S, OUT)
    full = np.concatenate(outs, axis=0)           # (B, S, OUT)
    return np.ascontiguousarray(full.transpose(0, 2, 1)).astype(np.float32)
